# revision 1
# baseline (speedup 1.0000x reference)
"""Cross-attention kernel for TRN2, 8 NeuronCores.

Sharding: core (b, g) = batch b (4) x head-group g (2 groups of 4 heads).
Each core computes q/k/v projections for its 4 heads on its batch, full
T x (T+2) attention for those heads, and a partial output projection
(contribution of its 4 heads to out = attn @ Wo.T). Host sums the two
partials per batch and adds the constant (bo + Wo @ bv) term.

Math notes (vs reference):
  - 1/sqrt(Dh) folded into Wq/bq host-side.
  - tanh(g) folded into the advisory-token stream host-side
    (hpTs = hp * tanh(g), bkad = bk * tanh(g)).
  - softmax computed without max-subtraction (scores are O(5), exp is
    safe in fp32/bf16 range for this data distribution).
  - v-bias handled exactly on host: since rows of softmax sum to 1,
    its contribution to the output is the constant Wo @ bv.
  - all matmuls in bf16 with fp32 PSUM accumulation.
  - 1/rowsum computed as exp(-ln(sum)) on the scalar engine (same ACT
    table set as the softmax exp), broadcast across partitions with a
    rank-1 bf16 matmul.

On-chip layout (per core):
  xT  [DIM, T]   x transposed (d on partitions) -> rhs/lhsT of projections
  qT,kT [dh, T]  per head: head-dim on partitions
  scores_T [s, t] : s (keys) on partitions -> softmax sum over s via
                    ones-vector matmul; AV matmul consumes scores as rhs.
  oT [dh, t] accumulated in PSUM, normalized by 1/rowsum via broadcast
  o_proj: out[t, dm] = sum_heads oT.T @ WoT slice.
"""

import math
import numpy as np
import ml_dtypes

import concourse.bass as bass
import concourse.mybir as mybir
import concourse.tile as tile
from concourse import bacc
from concourse.bass_utils import run_bass_kernel_spmd

BF16 = mybir.dt.bfloat16
F32 = mybir.dt.float32
AFT = mybir.ActivationFunctionType

P = 128
B, T, DIM = 4, 2048, 1024
NH, DH = 8, 128
HPG = 4              # heads per core
GD = HPG * DH        # 512 out-dims per core
KC = DIM // P        # 8 contraction chunks of the model dim
TT = 512             # t tile for attention
NT = T // TT         # 4 t tiles
NTC = T // P         # 16 t chunks of 128 (v layout, o-proj)
SFC = T // P         # 16 full s-chunks (key chunks of 128)

_CACHE = {}


def _build():
    nc = bacc.Bacc(
        "TRN2", target_bir_lowering=False, debug=False, enable_asserts=False
    )

    d = {}
    for name, shape, dt in [
        ("xT", [DIM, T], BF16),
        ("wqT", [DIM, GD], BF16),
        ("wkT", [DIM, GD], BF16),
        ("wvT", [DIM, GD], BF16),
        ("woT", [GD, DIM], BF16),
        ("bqv", [P, HPG], F32),
        ("bkv", [P, HPG], F32),
        ("bkad", [P, HPG], F32),
        ("hpT", [DIM, 2], BF16),
        ("hpTs", [DIM, 2], BF16),
    ]:
        d[name] = nc.dram_tensor(name, shape, dt, kind="ExternalInput").ap()
    out_ap = nc.dram_tensor("out", [T, DIM], F32, kind="ExternalOutput").ap()

    with tile.TileContext(nc) as tc:
        with (
            tc.tile_pool(name="big", bufs=1) as big,
            tc.tile_pool(name="expp", bufs=6) as expp,
            tc.tile_pool(name="accp", bufs=4) as accp,
            tc.tile_pool(name="bcap", bufs=2) as bcap,
            tc.tile_pool(name="rcp", bufs=2) as rcp,
            tc.tile_pool(name="ostg", bufs=3) as ostg,
        ):
            # ---- persistent SBUF residents ----
            xt = big.tile([P, KC, T], BF16)
            wq = big.tile([P, KC, GD], BF16)
            wk = big.tile([P, KC, GD], BF16)
            wv = big.tile([P, KC, GD], BF16)
            wo = big.tile([P, HPG, DIM], BF16)
            bq_s = big.tile([P, HPG], F32)
            bk_s = big.tile([P, HPG], F32)
            bkad_s = big.tile([P, HPG], F32)
            hpt = big.tile([P, KC, 2], BF16)
            hpts = big.tile([P, KC, 2], BF16)
            qt = big.tile([P, HPG, T], BF16)
            kt = big.tile([P, HPG, T + 2], BF16)
            vsb = big.tile([P, NTC, GD], BF16)
            vad = big.tile([2, GD], BF16)
            ot = big.tile([P, HPG, T], BF16)
            ones_s = big.tile([P, 1], BF16)   # partition-sum lhsT
            ones_b = big.tile([1, P], BF16)   # broadcast lhsT

            # ---- input DMAs: first q-proj needs wq[0] + xt[0] ----
            xTd = d["xT"].rearrange("(c p) t -> p c t", p=P)
            wqd = d["wqT"].rearrange("(c p) f -> p c f", p=P)
            wkd = d["wkT"].rearrange("(c p) f -> p c f", p=P)
            wvd = d["wvT"].rearrange("(c p) f -> p c f", p=P)
            wod = d["woT"].rearrange("(c p) f -> p c f", p=P)
            # split input traffic across the sync (HWDGE) and gpsimd (SWDGE)
            # queues so k/v weights don't queue behind the 4MB xT load.
            nc.gpsimd.dma_start(wq[:, 0, :], wqd[:, 0, :])
            for q4 in range(4):
                qsl = slice(q4 * TT, (q4 + 1) * TT)
                nc.sync.dma_start(xt[:, 0, qsl], xTd[:, 0, qsl])
            for c in range(1, KC):
                nc.gpsimd.dma_start(wq[:, c, :], wqd[:, c, :])
                nc.gpsimd.dma_start(wk[:, c - 1, :], wkd[:, c - 1, :])
                for q4 in range(4):
                    qsl = slice(q4 * TT, (q4 + 1) * TT)
                    nc.sync.dma_start(xt[:, c, qsl], xTd[:, c, qsl])
            nc.gpsimd.dma_start(wk[:, KC - 1, :], wkd[:, KC - 1, :])
            nc.gpsimd.dma_start(bq_s[:], d["bqv"][:])
            nc.gpsimd.dma_start(bk_s[:], d["bkv"][:])
            nc.gpsimd.dma_start(bkad_s[:], d["bkad"][:])
            nc.gpsimd.dma_start(hpt[:], d["hpT"].rearrange("(c p) t -> p c t", p=P))
            nc.gpsimd.dma_start(hpts[:], d["hpTs"].rearrange("(c p) t -> p c t", p=P))
            for c in range(KC):
                nc.gpsimd.dma_start(wv[:, c, :], wvd[:, c, :])
            for c in range(HPG):
                nc.sync.dma_start(wo[:, c, :], wod[:, c, :])
            nc.vector.memset(ones_s[:], 1.0)
            nc.vector.memset(ones_b[:], 1.0)

            # ---- phase A: projections ----
            with tc.tile_pool(name="psP", bufs=4, space="PSUM") as psP:

                def qk_proj(h, w, bias, dst):
                    for tti in range(NT):
                        ts = slice(tti * TT, (tti + 1) * TT)
                        ps = psP.tile([P, TT], F32, tag="ppsum",
                                      name=f"pp_{h}_{tti}")
                        for c in range(KC):
                            nc.tensor.matmul(
                                ps[:],
                                w[:, c, h * P : (h + 1) * P],
                                xt[:, c, ts],
                                start=(c == 0),
                                stop=(c == KC - 1),
                            )
                        nc.scalar.activation(
                            dst[:, h, ts], ps[:], AFT.Identity,
                            bias=bias[:, h : h + 1],
                        )

                def kad_proj(h):
                    ps = psP.tile([P, 2], F32, tag="adsum", name=f"kad_{h}")
                    for c in range(KC):
                        nc.tensor.matmul(
                            ps[:],
                            wk[:, c, h * P : (h + 1) * P],
                            hpts[:, c, :],
                            start=(c == 0),
                            stop=(c == KC - 1),
                        )
                    nc.scalar.activation(
                        kt[:, h, T : T + 2], ps[:], AFT.Identity,
                        bias=bkad_s[:, h : h + 1],
                    )

                # q/k for all heads first (their DMAs land first); v last so
                # its PSUM tiles don't head-of-line-block q/k behind the wv
                # DMA. Attention (h0) starts on scores while v finishes.
                for h in range(HPG):
                    qk_proj(h, wq, bq_s, qt)
                    qk_proj(h, wk, bk_s, kt)
                    kad_proj(h)
                for tci in range(NTC):
                    ps = psP.tile([P, GD], F32, tag="ppsum", name=f"vp_{tci}")
                    for c in range(KC):
                        nc.tensor.matmul(
                            ps[:],
                            xt[:, c, tci * P : (tci + 1) * P],
                            wv[:, c, :],
                            start=(c == 0),
                            stop=(c == KC - 1),
                        )
                    nc.vector.tensor_copy(vsb[:, tci, :], ps[:])
                vps = psP.tile([2, GD], F32, tag="adsum", name="vad_ps")
                for c in range(KC):
                    nc.tensor.matmul(
                        vps[:], hpt[:, c, :], wv[:, c, :],
                        start=(c == 0), stop=(c == KC - 1),
                    )
                nc.vector.tensor_copy(vad[:], vps[:])

            # ---- phase B: attention, per (head, t-tile) ----
            with (
                tc.tile_pool(name="psS", bufs=3, space="PSUM") as psS,
                tc.tile_pool(name="psO", bufs=3, space="PSUM") as psO,
                tc.tile_pool(name="psM", bufs=2, space="PSUM") as psM,
            ):
                for h in range(HPG):
                    hs = slice(h * P, (h + 1) * P)
                    for tti in range(NT):
                        ts = slice(tti * TT, (tti + 1) * TT)
                        ops = psO.tile([P, TT], F32, tag="avacc",
                                       name=f"av_{h}_{tti}")
                        acc = accp.tile([P, TT], BF16, tag="sumacc",
                                        name=f"acc_{h}_{tti}")
                        for sc in range(SFC):
                            sps = psS.tile([P, TT], F32, tag="scores",
                                           name=f"s_{h}_{tti}_{sc}")
                            nc.tensor.matmul(
                                sps[:], kt[:, h, sc * P : (sc + 1) * P],
                                qt[:, h, ts], start=True, stop=True,
                            )
                            et = expp.tile([P, TT], BF16, tag="exp",
                                           name=f"e_{h}_{tti}_{sc}")
                            nc.scalar.activation(et[:], sps[:], AFT.Exp)
                            if sc == 0:
                                nc.vector.tensor_copy(acc[:], et[:])
                            else:
                                nc.vector.tensor_add(acc[:], acc[:], et[:])
                            nc.tensor.matmul(
                                ops[:], vsb[:, sc, hs], et[:],
                                start=(sc == 0), stop=False,
                            )
                        # advisory-token tail (2 extra keys)
                        tps = psM.tile([2, TT], F32, tag="small",
                                       name=f"st_{h}_{tti}")
                        nc.tensor.matmul(tps[:], kt[:, h, T : T + 2],
                                         qt[:, h, ts], start=True, stop=True)
                        ett = expp.tile([2, TT], BF16, tag="exptail",
                                        name=f"et_{h}_{tti}")
                        nc.scalar.activation(ett[:], tps[:], AFT.Exp)
                        nc.vector.tensor_add(acc[0:2, :], acc[0:2, :], ett[:])
                        nc.tensor.matmul(ops[:], vad[:, hs], ett[:],
                                         start=False, stop=True)
                        # denominators: partition-sum, 1/x, broadcast
                        smp = psM.tile([1, TT], F32, tag="small",
                                       name=f"sm_{h}_{tti}")
                        nc.tensor.matmul(smp[:], ones_s[:], acc[:],
                                         start=True, stop=True)
                        rcf = rcp.tile([1, TT], F32, tag="recipf",
                                       name=f"rf_{h}_{tti}")
                        nc.vector.reciprocal_approx_fast(rcf[:], smp[:])
                        rc = rcp.tile([1, TT], BF16, tag="recip",
                                      name=f"rc_{h}_{tti}")
                        nc.vector.tensor_copy(rc[:], rcf[:])
                        bps = psM.tile([P, TT], F32, tag="small",
                                       name=f"bc_{h}_{tti}")
                        nc.tensor.matmul(bps[:], ones_b[:], rc[:],
                                         start=True, stop=True)
                        bsb = bcap.tile([P, TT], F32, tag="bcast",
                                        name=f"bs_{h}_{tti}")
                        nc.scalar.activation(bsb[:], bps[:], AFT.Copy)
                        # normalize + downcast into persistent oT
                        nc.vector.tensor_mul(ot[:, h, ts], ops[:], bsb[:])

            # ---- phase C: partial output projection ----
            with tc.tile_pool(name="psC", bufs=3, space="PSUM") as psC:
                for tci in range(NTC):
                    stg = ostg.tile([P, DIM], F32, tag="ostage",
                                    name=f"o_{tci}")
                    for half in range(2):
                        cps = psC.tile([P, 512], F32, tag="opsum",
                                       name=f"op_{tci}_{half}")
                        for c in range(HPG):
                            nc.tensor.matmul(
                                cps[:],
                                ot[:, c, tci * P : (tci + 1) * P],
                                wo[:, c, half * 512 : (half + 1) * 512],
                                start=(c == 0),
                                stop=(c == HPG - 1),
                            )
                        nc.vector.tensor_copy(
                            stg[:, half * 512 : (half + 1) * 512], cps[:]
                        )
                    nc.sync.dma_start(out_ap[tci * P : (tci + 1) * P, :], stg[:])

    nc.compile()
    return nc


def _get_nc():
    if "nc" not in _CACHE:
        _CACHE["nc"] = _build()
    return _CACHE["nc"]


def kernel(x, h, p, Wq, bq, Wk, bk, Wv, bv, Wo, bo, g, **_):
    x = np.asarray(x, np.float32)
    h = np.asarray(h, np.float32)
    p = np.asarray(p, np.float32)
    Wq = np.asarray(Wq, np.float32)
    bq = np.asarray(bq, np.float32)
    Wk = np.asarray(Wk, np.float32)
    bk = np.asarray(bk, np.float32)
    Wv = np.asarray(Wv, np.float32)
    bv = np.asarray(bv, np.float32)
    Wo = np.asarray(Wo, np.float32)
    bo = np.asarray(bo, np.float32)
    g = np.asarray(g, np.float32)

    nc = _get_nc()
    bf = ml_dtypes.bfloat16
    s = 1.0 / math.sqrt(DH)
    gt = float(np.tanh(g[0]))
    hp = np.concatenate([h, p], axis=1)  # [B, 2, DIM]

    per_group = []
    for gi in range(2):
        sl = slice(gi * GD, (gi + 1) * GD)
        per_group.append({
            "wqT": np.ascontiguousarray((Wq[sl] * s).T).astype(bf),
            "wkT": np.ascontiguousarray(Wk[sl].T).astype(bf),
            "wvT": np.ascontiguousarray(Wv[sl].T).astype(bf),
            "woT": np.ascontiguousarray(Wo[:, sl].T).astype(bf),
            "bqv": np.ascontiguousarray((bq[sl] * s).reshape(HPG, P).T,
                                        dtype=np.float32),
            "bkv": np.ascontiguousarray(bk[sl].reshape(HPG, P).T,
                                        dtype=np.float32),
            "bkad": np.ascontiguousarray((bk[sl] * gt).reshape(HPG, P).T,
                                         dtype=np.float32),
        })

    in_maps = []
    for b in range(B):
        xTb = np.ascontiguousarray(x[b].T).astype(bf)
        hpTb = np.ascontiguousarray(hp[b].T).astype(bf)
        hpTsb = np.ascontiguousarray((hp[b] * gt).T).astype(bf)
        for gi in range(2):
            m = dict(per_group[gi])
            m["xT"] = xTb
            m["hpT"] = hpTb
            m["hpTs"] = hpTsb
            in_maps.append(m)

    _CACHE["last_in_maps"] = in_maps
    res = run_bass_kernel_spmd(nc, in_maps, list(range(8)))
    outs = res.results

    const = (bo + Wo @ bv).astype(np.float32)
    out = np.empty((B, T, DIM), np.float32)
    for b in range(B):
        out[b] = outs[2 * b]["out"] + outs[2 * b + 1]["out"] + const
    return out

